# revision 1
# baseline (speedup 1.0000x reference)
"""Multi-Head Latent Attention (MLA) forward on 8 Trainium2 NeuronCores.

Sharding: tensor-parallel over heads (16 heads -> 2 per core). Each core:
  - transposes x to feature-major layout on the PE (fp32r),
  - computes q projections for its heads plus a 1/8 column slice of the
    latent-kv encoding; per-block AllGather assembles the full latent
    (hidden behind compute),
  - RMS-norms the latent (norm weight folded into wkv_b on host, per-token
    scale applied after the up-projection),
  - applies RoPE with host-precomputed cos/sin tables,
  - runs causal attention for its 2 heads in transposed-score layout
    (scores St[k, q]; softmax without max subtraction - scores are O(1)),
  - per-batch AllToAll exchanges head outputs so each core holds all
    features for a token slice, then computes that slice of the wo
    projection; batch 0's exchange + wo overlap batch 1's compute.
Output slices are disjoint; the host just concatenates them.
"""
import sys

if "/opt/trn_rl_repo" not in sys.path:
    sys.path.insert(0, "/opt/trn_rl_repo")

import numpy as np
import concourse.bacc as bacc
import concourse.mybir as mybir
from concourse import tile
from concourse.masks import make_identity
from concourse.bass_utils import run_bass_kernel_spmd

H, NOPE, ROPE, VD, KVR, QKD = 16, 128, 64, 128, 512, 192
B, T, D = 2, 2048, 2048
NCORES, HPC, BLK = 8, 2, 512
KVC = KVR + ROPE  # 576 latent+rope columns
KVS = KVC // NCORES  # 72-column slice per core
W1N = HPC * QKD + KVS  # 456 projection columns per core
f32 = mybir.dt.float32
f32r = mybir.dt.float32r
EXP = mybir.ActivationFunctionType.Exp
LN = mybir.ActivationFunctionType.Ln
SQUARE = mybir.ActivationFunctionType.Square


def r32(ap):
    return ap.bitcast(f32r)


def _patch_act_tables():
    """Make the act-table-load pass serve Exp/Ln/Square from the one set that
    contains them all (natural_log_exp_and_others), so interleaved activations
    don't thrash table loads. Indices into act_info.json must be preserved, so
    the shadowing single-function sets are emptied in place, not removed."""
    import concourse.bacc as _bacc

    orig = _bacc.get_activation_tables
    if getattr(_bacc, "_mla_act_patch", False):
        return
    _bacc._mla_act_patch = True

    def patched(arch):
        d = dict(orig(arch))
        if "natural_log_exp_and_others" in d:
            for name in ("exp_and_others", "natural_log", "exp_and_friends"):
                if name in d:
                    d[name] = set()
        return d

    _bacc.get_activation_tables = patched


def build_program():
    _patch_act_tables()
    nc = bacc.Bacc("TRN2", target_bir_lowering=False, debug=False, num_devices=NCORES)
    x_d = nc.dram_tensor("x", [B * T, D], f32, kind="ExternalInput")
    w1_d = nc.dram_tensor("w1", [D, W1N], f32, kind="ExternalInput")
    wb_d = nc.dram_tensor("wb", [KVR, HPC * (NOPE + VD)], f32, kind="ExternalInput")
    wo_d = nc.dram_tensor("wo", [H * VD, D], f32, kind="ExternalInput")
    cos_d = nc.dram_tensor("cos", [128, T], f32, kind="ExternalInput")
    sin_d = nc.dram_tensor("sin", [128, T], f32, kind="ExternalInput")
    out_d = nc.dram_tensor("out", [B, T // NCORES, D], f32, kind="ExternalOutput")

    RG = [list(range(NCORES))]

    with tile.TileContext(nc) as tc:
        with (
            tc.tile_pool(name="dram", bufs=1, space="DRAM") as dram,
            tc.tile_pool(name="const", bufs=1) as const,
            tc.tile_pool(name="wpool", bufs=1) as wpool,
            tc.tile_pool(name="kvpool", bufs=1) as kvpool,
            tc.tile_pool(name="xnpool", bufs=4) as xnpool,
            tc.tile_pool(name="work", bufs=1) as work,
            tc.tile_pool(name="wop", bufs=1) as wop,
            tc.tile_pool(name="ps", bufs=1, space="PSUM") as ps,
        ):
            y_in = [
                dram.tile([NCORES, HPC * VD, 256], f32, name=f"y_in{b}")
                for b in range(B)
            ]
            y_out = [
                dram.tile([NCORES, HPC * VD, 256], f32, name=f"y_out{b}")
                for b in range(B)
            ]
            ag_in = [
                [dram.tile([KVS, BLK], f32, name=f"ag_in{b}_{q}") for q in range(4)]
                for b in range(B)
            ]
            ag_out = [
                [dram.tile([KVC, BLK], f32, name=f"ag_out{b}_{q}") for q in range(4)]
                for b in range(B)
            ]

            ident_f = const.tile([128, 128], f32, tag="ident_f")
            make_identity(nc, ident_f)
            ident = const.tile([128, 128], f32r, tag="ident")
            nc.vector.tensor_copy(ident[:], ident_f[:])
            ones_f = const.tile([128, 1], f32, tag="ones_f")
            nc.gpsimd.memset(ones_f[:], 1.0)
            ones = const.tile([128, 1], f32r, tag="ones")
            nc.vector.tensor_copy(ones[:], ones_f[:])
            onesrow_f = const.tile([1, 128], f32, tag="onesrow_f")
            nc.gpsimd.memset(onesrow_f[:], 1.0)
            onesrow = const.tile([1, 128], f32r, tag="onesrow")
            nc.vector.tensor_copy(onesrow[:], onesrow_f[:])
            eps = const.tile([1, 1], f32, tag="eps")
            nc.gpsimd.memset(eps[:], 1e-6)
            w1_sb = wpool.tile([128, 16, W1N], f32r, tag="w1")
            nc.sync.dma_start(
                w1_sb[:], w1_d[:].rearrange("(kc p) m -> p kc m", p=128).bitcast(f32r)
            )
            wb_sb = wpool.tile([128, 4, 512], f32r, tag="wb")
            nc.sync.dma_start(
                wb_sb[:], wb_d[:].rearrange("(kc p) m -> p kc m", p=128).bitcast(f32r)
            )

            # per-batch persistent kv staging (slots reused across batches)
            def alloc_kv():
                knope = [
                    kvpool.tile(
                        [NOPE, T], f32r, tag=f"knope{h}", bufs=1, name=f"knope{h}"
                    )
                    for h in range(2)
                ]
                vnat = [
                    kvpool.tile(
                        [128, 16, VD], f32r, tag=f"vnat{h}", bufs=1, name=f"vnat{h}"
                    )
                    for h in range(2)
                ]
                krope = kvpool.tile([ROPE, T], f32r, tag="krope", bufs=1)
                return knope, vnat, krope

            def stage_a1(b, qc):
                """x transpose + fused q/kvc-slice projection + q rope + AG."""
                row0 = b * T + qc * BLK
                tok = slice(qc * BLK, (qc + 1) * BLK)
                csb = work.tile([128, BLK], f32, tag="csb", bufs=3)
                ssb = work.tile([128, BLK], f32, tag="ssb", bufs=3)
                nc.sync.dma_start(csb[:], cos_d[:, tok])
                nc.sync.dma_start(ssb[:], sin_d[:, tok])

                # fused transpose + projection, xT chunks streamed
                pp = [
                    ps.tile([128, BLK], f32, tag="proj", bufs=4, name="projp")
                    for _ in range(4)
                ]
                for k4 in range(4):
                    xns = []
                    for tt in range(4):
                        xn = xnpool.tile([128, BLK], f32r, tag="xn", bufs=4)
                        nc.sync.dma_start(
                            xn[:],
                            x_d[
                                row0 + tt * 128 : row0 + (tt + 1) * 128,
                                k4 * 512 : (k4 + 1) * 512,
                            ].bitcast(f32r),
                        )
                        xns.append(xn)
                    for kk in range(4):
                        kc = k4 * 4 + kk
                        pst = ps.tile([128, BLK], f32, tag="xps", bufs=1)
                        for tt in range(4):
                            nc.tensor.transpose(
                                r32(pst[:, tt * 128 : (tt + 1) * 128]),
                                r32(xns[tt][:, kk * 128 : (kk + 1) * 128]),
                                r32(ident[:]),
                            )
                        xTc = work.tile([128, BLK], f32r, tag="xTc", bufs=6)
                        nc.vector.tensor_copy(xTc[:], pst[:])
                        for mc in range(4):
                            m0 = mc * 128
                            m1 = min(m0 + 128, W1N)
                            nc.tensor.matmul(
                                pp[mc][: m1 - m0, :],
                                r32(w1_sb[:, kc, m0:m1]),
                                r32(xTc[:]),
                                start=(kc == 0),
                                stop=(kc == 15),
                            )

                # q: nope chunks straight, rope chunk roped
                qfT = work.tile([128, 4, BLK], f32r, tag="qfT", bufs=3)
                nc.vector.tensor_copy(qfT[:, 0, :], pp[0][:])
                nc.vector.tensor_copy(qfT[:, 1, :], pp[1][:])
                rot = work.tile([128, BLK], f32r, tag="rot", bufs=2)
                pq = pp[2]
                for hh in range(2):
                    r0 = hh * 64
                    nc.vector.tensor_scalar_mul(
                        rot[r0 : r0 + 32, :], pq[r0 + 32 : r0 + 64, :], -1.0
                    )
                    nc.vector.tensor_copy(
                        rot[r0 + 32 : r0 + 64, :], pq[r0 : r0 + 32, :]
                    )
                nc.vector.tensor_mul(out=qfT[:, 2, :], in0=pq[:], in1=csb[:])
                nc.vector.tensor_mul(out=rot[:], in0=rot[:], in1=ssb[:])
                nc.vector.tensor_add(out=qfT[:, 2, :], in0=qfT[:, 2, :], in1=rot[:])
                # h1 roped rows 64:128 -> chunk 3 rows 0:64 (32-wide moves)
                nc.vector.tensor_copy(qfT[0:32, 3, :], qfT[64:96, 2, :])
                nc.vector.tensor_copy(qfT[32:64, 3, :], qfT[96:128, 2, :])

                # my kvc slice -> dram -> allgather
                kvcm = work.tile([KVS, BLK], f32, tag="kvcm", bufs=1)
                nc.vector.tensor_copy(kvcm[:], pp[3][:KVS, :])
                nc.sync.dma_start(ag_in[b][qc][:], kvcm[:])
                nc.gpsimd.collective_compute(
                    "AllGather",
                    mybir.AluOpType.bypass,
                    replica_groups=RG,
                    ins=[ag_in[b][qc].opt()],
                    outs=[ag_out[b][qc].opt()],
                )
                return qfT, csb, ssb

            def stage_a2(b, qc, knope, vnat, krope, csb, ssb):
                """post-AG: rms norm, kv up-projection, k rope."""
                tok = slice(qc * BLK, (qc + 1) * BLK)
                latent = work.tile([128, 4, BLK], f32r, tag="latent", bufs=1)
                nc.sync.dma_start(
                    latent[:],
                    ag_out[b][qc][: KVR, :]
                    .rearrange("(kc p) t -> p kc t", p=128)
                    .bitcast(f32r),
                )
                kraw = work.tile([ROPE, BLK], f32, tag="kraw", bufs=2)
                nc.sync.dma_start(kraw[:], ag_out[b][qc][KVR:, :])

                # sum of squares over latent dims (ACT square + PE ones-mm)
                ssq = ps.tile([1, BLK], f32, tag="xps", bufs=1, name="ssq")
                for i in range(4):
                    sqc = work.tile([128, BLK], f32r, tag="sqc", bufs=1)
                    nc.scalar.activation(sqc[:], latent[:, i, :], SQUARE)
                    nc.tensor.matmul(
                        ssq[:], ones[:], sqc[:], start=(i == 0), stop=(i == 3)
                    )
                # rms scale: 1/sqrt(ssq/512+eps) = exp(-0.5*ln(.))
                lnrow = work.tile([1, BLK], f32, tag="lnrow", bufs=2)
                nc.scalar.activation(lnrow[:], ssq[:], LN, bias=eps[:], scale=1.0 / KVR)
                invrow = work.tile([1, BLK], f32r, tag="invrow", bufs=2)
                nc.scalar.activation(invrow[:], lnrow[:], EXP, scale=-0.5)
                invbc_ps = ps.tile([128, BLK], f32, tag="xps", bufs=1, name="invbc_ps")
                nc.tensor.matmul(invbc_ps[:], onesrow[:], invrow[:])
                invbc = work.tile([128, BLK], f32, tag="invbc", bufs=2)
                nc.vector.tensor_copy(invbc[:], invbc_ps[:])

                # k rope from gathered raw rows
                rot = work.tile([128, BLK], f32r, tag="rot", bufs=2)
                kr = krope[:, tok]
                nc.vector.tensor_scalar_mul(rot[0:32, :], kraw[32:64, :], -1.0)
                nc.vector.tensor_copy(rot[32:64, :], kraw[0:32, :])
                nc.vector.tensor_mul(out=kr, in0=kraw[:], in1=csb[0:64, :])
                nc.vector.tensor_mul(
                    out=rot[0:64, :], in0=rot[0:64, :], in1=ssb[0:64, :]
                )
                nc.vector.tensor_add(out=kr, in0=kr, in1=rot[0:64, :])

                # kv up-projection + normalize; v transposed to natural
                for mc in range(4):  # [h0 nope, h0 v, h1 nope, h1 v]
                    h = mc // 2
                    pkv = ps.tile([128, BLK], f32, tag="proj", bufs=4)
                    for kc in range(4):
                        nc.tensor.matmul(
                            pkv[:],
                            r32(wb_sb[:, kc, mc * 128 : (mc + 1) * 128]),
                            r32(latent[:, kc, :]),
                            start=(kc == 0),
                            stop=(kc == 3),
                        )
                    if mc % 2 == 0:
                        nc.vector.tensor_mul(
                            out=knope[h][:, tok], in0=pkv[:], in1=invbc[:]
                        )
                    else:
                        vuT = work.tile([128, BLK], f32r, tag="vuT", bufs=1)
                        nc.vector.tensor_mul(out=vuT[:], in0=pkv[:], in1=invbc[:])
                        pvt = ps.tile([128, BLK], f32, tag="xps", bufs=1)
                        for tt in range(4):
                            nc.tensor.transpose(
                                r32(pvt[:, tt * 128 : (tt + 1) * 128]),
                                r32(vuT[:, tt * 128 : (tt + 1) * 128]),
                                r32(ident[:]),
                            )
                        for tt in range(4):
                            nc.vector.tensor_copy(
                                vnat[h][:, qc * 4 + tt, :],
                                pvt[:, tt * 128 : (tt + 1) * 128],
                            )

            def stage_b(b, qc, qfT, knope, vnat, krope):
                """causal attention for one q-chunk, both heads."""
                n_kt = 4 * (qc + 1)
                for h in range(2):
                    yacc = ps.tile([VD, BLK], f32, tag="yacc", bufs=1)
                    acc = work.tile([128, BLK], f32r, tag="acc", bufs=1)
                    qrope = qfT[0:64, 2 + h, :]
                    for kt in range(n_kt):
                        ks = slice(kt * 128, (kt + 1) * 128)
                        st = ps.tile([128, BLK], f32, tag="st", bufs=2)
                        nc.tensor.matmul(
                            st[:],
                            r32(knope[h][:, ks]),
                            r32(qfT[:, h, :]),
                            start=True,
                            stop=False,
                        )
                        nc.tensor.matmul(
                            st[:], r32(krope[:, ks]), r32(qrope), start=False, stop=True
                        )
                        est = work.tile([128, BLK], f32r, tag="est", bufs=2)
                        nc.scalar.activation(est[:], st[:], EXP)
                        if (kt + 1) * 128 > qc * BLK:
                            nc.gpsimd.affine_select(
                                out=est[:],
                                in_=est[:],
                                compare_op=mybir.AluOpType.is_ge,
                                fill=0.0,
                                base=qc * BLK - kt * 128,
                                pattern=[[1, BLK]],
                                channel_multiplier=-1,
                            )
                        nc.tensor.matmul(
                            yacc[:],
                            r32(vnat[h][:, kt, :]),
                            r32(est[:]),
                            start=(kt == 0),
                            stop=(kt == n_kt - 1),
                        )
                        if kt == 0:
                            nc.gpsimd.tensor_copy(acc[:], est[:])
                        else:
                            nc.gpsimd.tensor_add(out=acc[:], in0=acc[:], in1=est[:])

                    sums = ps.tile([1, BLK], f32, tag="st", bufs=2)
                    nc.tensor.matmul(sums[:], ones[:], acc[:])
                    lnr = work.tile([1, BLK], f32, tag="lnrow", bufs=2)
                    nc.scalar.activation(lnr[:], sums[:], LN)
                    sinvrow = work.tile([1, BLK], f32r, tag="invrow", bufs=2)
                    nc.scalar.activation(sinvrow[:], lnr[:], EXP, scale=-1.0)
                    sbc_ps = ps.tile([128, BLK], f32, tag="st", bufs=2, name="sbc_ps")
                    nc.tensor.matmul(sbc_ps[:], onesrow[:], sinvrow[:])
                    sinv = work.tile([128, BLK], f32, tag="sinv", bufs=1)
                    nc.vector.tensor_copy(sinv[:], sbc_ps[:])
                    ysb = work.tile([VD, BLK], f32, tag="ysb", bufs=2)
                    nc.vector.tensor_mul(out=ysb[:], in0=yacc[:], in1=sinv[:])
                    for jj in range(2):
                        nc.sync.dma_start(
                            y_in[b][qc * 2 + jj, h * VD : (h + 1) * VD, :],
                            ysb[:, jj * 256 : (jj + 1) * 256],
                        )

            def emit_a2a(b):
                nc.gpsimd.collective_compute(
                    "AllToAll",
                    mybir.AluOpType.bypass,
                    replica_groups=RG,
                    ins=[y_in[b].opt()],
                    outs=[y_out[b].opt()],
                )

            def emit_wo(b):
                """wo projection for this batch's gathered token slice."""
                a2a = wop.tile([128, 16, 256], f32r, tag="a2a", bufs=1, name="a2a")
                for kc in range(16):
                    nc.sync.dma_start(
                        a2a[:, kc, :],
                        y_out[b][
                            kc // 2, (kc % 2) * 128 : (kc % 2) * 128 + 128, :
                        ].bitcast(f32r),
                    )
                for n in range(4):
                    pouts = [
                        ps.tile([128, 512], f32, tag="proj", bufs=4, name="outp")
                        for _ in range(2)
                    ]
                    for kc in range(16):
                        wt = wop.tile([128, 512], f32r, tag="wt", bufs=3)
                        nc.sync.dma_start(
                            wt[:],
                            wo_d[
                                kc * 128 : (kc + 1) * 128, n * 512 : (n + 1) * 512
                            ].bitcast(f32r),
                        )
                        for tt in range(2):
                            nc.tensor.matmul(
                                pouts[tt][:],
                                r32(a2a[:, kc, tt * 128 : (tt + 1) * 128]),
                                r32(wt[:]),
                                start=(kc == 0),
                                stop=(kc == 15),
                            )
                    for tt in range(2):
                        osb = wop.tile([128, 512], f32, tag="osb", bufs=2)
                        nc.vector.tensor_copy(osb[:], pouts[tt][:])
                        nc.sync.dma_start(
                            out_d[
                                b, tt * 128 : (tt + 1) * 128, n * 512 : (n + 1) * 512
                            ],
                            osb[:],
                        )

            # ---- software-pipelined schedule ----
            for b in range(B):
                knope, vnat, krope = alloc_kv()
                st_a = {}
                st_a[0] = stage_a1(b, 0)
                st_a[1] = stage_a1(b, 1)
                stage_a2(b, 0, knope, vnat, krope, st_a[0][1], st_a[0][2])
                st_a[2] = stage_a1(b, 2)
                stage_a2(b, 1, knope, vnat, krope, st_a[1][1], st_a[1][2])
                stage_b(b, 0, st_a[0][0], knope, vnat, krope)
                st_a[3] = stage_a1(b, 3)
                stage_a2(b, 2, knope, vnat, krope, st_a[2][1], st_a[2][2])
                stage_b(b, 1, st_a[1][0], knope, vnat, krope)
                stage_a2(b, 3, knope, vnat, krope, st_a[3][1], st_a[3][2])
                stage_b(b, 2, st_a[2][0], knope, vnat, krope)
                stage_b(b, 3, st_a[3][0], knope, vnat, krope)
                emit_a2a(b)
                emit_wo(b)

    nc.compile()
    return nc


def host_prep(x, wq, wkv_a, wkv_b, wo, kv_norm_w):
    scale = np.float32(QKD ** -0.5)
    inv = (1.0 / (10000.0 ** (np.arange(0, ROPE, 2, dtype=np.float32) / ROPE))).astype(
        np.float32
    )
    f = np.outer(np.arange(T, dtype=np.float32), inv)
    cos32 = np.cos(f).T.astype(np.float32)
    sin32 = np.sin(f).T.astype(np.float32)
    cos128 = np.ascontiguousarray(np.concatenate([cos32] * 4, 0))
    sin128 = np.ascontiguousarray(np.concatenate([sin32] * 4, 0))
    wkv_bw = (wkv_b * kv_norm_w[:, None]).astype(np.float32)
    x2 = np.ascontiguousarray(x.reshape(B * T, D))
    wo_c = np.ascontiguousarray(wo)
    wq_r = wq.reshape(D, H, QKD)

    in_maps = []
    for c in range(NCORES):
        h0 = HPC * c
        w1 = np.concatenate(
            [
                wq_r[:, h0, :NOPE] * scale,
                wq_r[:, h0 + 1, :NOPE] * scale,
                wq_r[:, h0, NOPE:] * scale,
                wq_r[:, h0 + 1, NOPE:] * scale,
                wkv_a[:, c * KVS : (c + 1) * KVS],
            ],
            axis=1,
        ).astype(np.float32)
        wb = np.ascontiguousarray(wkv_bw[:, h0 * (NOPE + VD) : (h0 + 2) * (NOPE + VD)])
        in_maps.append(
            {
                "x": x2,
                "w1": np.ascontiguousarray(w1),
                "wb": wb,
                "wo": wo_c,
                "cos": cos128,
                "sin": sin128,
            }
        )
    return in_maps


_NC = None


def kernel(x, wq, wkv_a, wkv_b, wo, kv_norm_w, _trace=False):
    global _NC
    if _NC is None:
        _NC = build_program()
    in_maps = host_prep(
        np.asarray(x, np.float32),
        np.asarray(wq, np.float32),
        np.asarray(wkv_a, np.float32),
        np.asarray(wkv_b, np.float32),
        np.asarray(wo, np.float32),
        np.asarray(kv_norm_w, np.float32),
    )
    res = run_bass_kernel_spmd(_NC, in_maps, list(range(NCORES)), trace=_trace)
    out = np.empty((B, T, D), np.float32)
    cw = T // NCORES
    for c in range(NCORES):
        oc = res.results[c]["out"]  # (B, 256, D)
        for b in range(B):
            out[b, c * cw : (c + 1) * cw, :] = oc[b]
    kernel.last_results = res
    return out



# revision 6
# speedup vs baseline: 2.1014x; 2.1014x over previous
"""Multi-Head Latent Attention (MLA) forward on 8 Trainium2 NeuronCores.

Sharding: tensor-parallel over heads (16 heads -> 2 per core). V2 design:
  - x is transposed to feature-major layout on the HOST (pure layout prep),
    so the device streams xT tiles straight into the projection matmuls --
    no PE transposes, no PSUM round-trips for x.
  - all matmul operands are bf16 (fp32 PSUM accumulation); softmax stats and
    normalization factors stay fp32 on the scalar engine.
  - each core computes q projections for its 2 heads plus a 1/8 column slice
    of the latent-kv encoding; per-block AllGather (Shared output buffers)
    assembles the full latent, hidden behind compute.
  - RMS norm: sum-of-squares via ACT square + PE ones-matmul; the latent is
    normalized BEFORE the up-projection (norm weight folded into wkv_b).
  - causal attention in transposed-score layout St[k, q], softmax without
    max subtraction (scores are O(1)); the two heads are interleaved with a
    one-kt software pipeline so the PE never waits on exp; the softmax
    denominator is accumulated with vector adds (fp32) and reduced across
    partitions with gpsimd partition_all_reduce -- no serial gpsimd chains.
  - per-batch AllToAll exchanges head outputs; each core computes a
    256-token slice of the wo projection. Batch 0's wo is emitted after
    batch 1's first projection stages so the PE keeps streaming through the
    collective latency.
Output slices are disjoint; the host just concatenates them.
"""
import sys

if "/opt/trn_rl_repo" not in sys.path:
    sys.path.insert(0, "/opt/trn_rl_repo")

import numpy as np
import ml_dtypes
import concourse.bacc as bacc
import concourse.mybir as mybir
from concourse import tile
from concourse import bass_isa
from concourse.masks import make_identity
from concourse.bass_utils import run_bass_kernel_spmd

H, NOPE, ROPE, VD, KVR, QKD = 16, 128, 64, 128, 512, 192
B, T, D = 2, 2048, 2048
NCORES, HPC, BLK = 8, 2, 512
KVC = KVR + ROPE  # 576 latent+rope columns
KVS = KVC // NCORES  # 72-column slice per core
W1N = HPC * QKD + KVS  # 456 projection columns per core
f32 = mybir.dt.float32
bf16 = mybir.dt.bfloat16
EXP = mybir.ActivationFunctionType.Exp
LN = mybir.ActivationFunctionType.Ln
SQUARE = mybir.ActivationFunctionType.Square
BF = ml_dtypes.bfloat16


def _patch_act_tables():
    """Make the act-table-load pass serve Exp/Ln/Square from the one set that
    contains them all (natural_log_exp_and_others), so interleaved activations
    don't thrash table loads."""
    import concourse.bacc as _bacc

    orig = _bacc.get_activation_tables
    if getattr(_bacc, "_mla_act_patch", False):
        return
    _bacc._mla_act_patch = True

    def patched(arch):
        d = dict(orig(arch))
        if "natural_log_exp_and_others" in d:
            for name in ("exp_and_others", "natural_log", "exp_and_friends"):
                if name in d:
                    d[name] = set()
        return d

    _bacc.get_activation_tables = patched


def build_program():
    _patch_act_tables()
    nc = bacc.Bacc("TRN2", target_bir_lowering=False, debug=False, num_devices=NCORES)
    xt_d = nc.dram_tensor("xt", [D, B * T], bf16, kind="ExternalInput")
    w1_d = nc.dram_tensor("w1", [D, W1N], bf16, kind="ExternalInput")
    wb_d = nc.dram_tensor("wb", [KVR, HPC * (NOPE + VD)], bf16, kind="ExternalInput")
    wo_d = nc.dram_tensor("wo", [H * VD, D], bf16, kind="ExternalInput")
    cos_d = nc.dram_tensor("cos", [128, T], bf16, kind="ExternalInput")
    sin_d = nc.dram_tensor("sin", [128, T], bf16, kind="ExternalInput")
    out_d = nc.dram_tensor("out", [B, T // NCORES, D], f32, kind="ExternalOutput")

    RG = [list(range(NCORES))]

    with tile.TileContext(nc) as tc:
        with (
            tc.tile_pool(name="dram", bufs=1, space="DRAM") as dram,
            tc.tile_pool(name="const", bufs=1) as const,
            tc.tile_pool(name="wpool", bufs=1) as wpool,
            tc.tile_pool(name="kvpool", bufs=1) as kvpool,
            tc.tile_pool(name="xpool", bufs=1) as xpool,
            tc.tile_pool(name="work", bufs=1) as work,
            tc.tile_pool(name="wop", bufs=1) as wop,
            tc.tile_pool(name="ps", bufs=1, space="PSUM") as ps,
        ):
            y_in = [
                dram.tile([NCORES, HPC * VD, 256], bf16, name=f"y_in{b}")
                for b in range(B)
            ]
            y_out = [
                dram.tile([NCORES, HPC * VD, 256], bf16, name=f"y_out{b}")
                for b in range(B)
            ]
            ag_in = [
                [dram.tile([KVS, BLK], bf16, name=f"ag_in{b}_{q}") for q in range(4)]
                for b in range(B)
            ]
            ag_out = [
                [
                    dram.tile(
                        [KVC, BLK], bf16, name=f"ag_out{b}_{q}", addr_space="Shared"
                    )
                    for q in range(4)
                ]
                for b in range(B)
            ]

            ident_f = const.tile([128, 128], f32, tag="ident_f")
            make_identity(nc, ident_f)
            ident = const.tile([128, 128], bf16, tag="ident")
            nc.vector.tensor_copy(ident[:], ident_f[:])
            ones_f = const.tile([128, 1], f32, tag="ones_f")
            nc.gpsimd.memset(ones_f[:], 1.0)
            ones = const.tile([128, 1], bf16, tag="ones")
            nc.vector.tensor_copy(ones[:], ones_f[:])
            eps = const.tile([1, 1], f32, tag="eps")
            nc.gpsimd.memset(eps[:], 1e-6)
            w1_sb = wpool.tile([128, 16, W1N], bf16, tag="w1")
            nc.sync.dma_start(w1_sb[:], w1_d[:].rearrange("(kc p) m -> p kc m", p=128))
            wb_sb = wpool.tile([128, 4, 512], bf16, tag="wb")
            nc.sync.dma_start(wb_sb[:], wb_d[:].rearrange("(kc p) m -> p kc m", p=128))
            cos_sb = wpool.tile([128, T], bf16, tag="cos")
            nc.sync.dma_start(cos_sb[:], cos_d[:])
            sin_sb = wpool.tile([128, T], bf16, tag="sin")
            nc.sync.dma_start(sin_sb[:], sin_d[:])

            # per-batch kv staging (bufs=2 -> batches rotate slots, no WAR stall)
            def alloc_kv():
                knope = [
                    kvpool.tile(
                        [NOPE, T], bf16, tag=f"knope{h}", bufs=2, name=f"knope{h}"
                    )
                    for h in range(2)
                ]
                vnat = [
                    kvpool.tile(
                        [128, 16, VD], bf16, tag=f"vnat{h}", bufs=2, name=f"vnat{h}"
                    )
                    for h in range(2)
                ]
                krope = kvpool.tile([ROPE, T], bf16, tag="krope", bufs=2)
                return knope, vnat, krope

            def stage_a1(b, qc):
                """xT-tile load + q/kvc-slice projection + q rope + AllGather."""
                tok = slice(qc * BLK, (qc + 1) * BLK)
                col0 = b * T + qc * BLK
                xTc = xpool.tile([128, 16, BLK], bf16, tag="xTc", bufs=2)
                nc.sync.dma_start(
                    xTc[:],
                    xt_d[:, col0 : col0 + BLK].rearrange("(kc p) t -> p kc t", p=128),
                )
                qfT = work.tile([128, 4, BLK], bf16, tag="qfT", bufs=3)
                csb = cos_sb[:, tok]
                ssb = sin_sb[:, tok]

                pps = []
                for mc in range(4):
                    m0 = mc * 128
                    m1 = min(m0 + 128, W1N)
                    pp = ps.tile([128, BLK], f32, tag="proj", bufs=2, name="projp")
                    pps.append(pp)
                    for kc in range(16):
                        nc.tensor.matmul(
                            pp[: m1 - m0, :],
                            w1_sb[:, kc, m0:m1],
                            xTc[:, kc, :],
                            start=(kc == 0),
                            stop=(kc == 15),
                        )
                    # post-process each chunk right away to free the PSUM slot
                    if mc < 2:
                        nc.scalar.copy(qfT[:, mc, :], pp[:])
                    elif mc == 2:
                        rot = work.tile([128, BLK], bf16, tag="rot", bufs=2)
                        for hh in range(2):
                            r0 = hh * 64
                            nc.vector.tensor_scalar_mul(
                                rot[r0 : r0 + 32, :], pp[r0 + 32 : r0 + 64, :], -1.0
                            )
                            nc.vector.tensor_copy(
                                rot[r0 + 32 : r0 + 64, :], pp[r0 : r0 + 32, :]
                            )
                        nc.vector.tensor_mul(out=qfT[:, 2, :], in0=pp[:], in1=csb)
                        nc.vector.tensor_mul(out=rot[:], in0=rot[:], in1=ssb)
                        nc.vector.tensor_add(
                            out=qfT[:, 2, :], in0=qfT[:, 2, :], in1=rot[:]
                        )
                        # h1 roped rows 64:128 -> chunk 3 rows 0:64
                        nc.vector.tensor_copy(qfT[0:32, 3, :], qfT[64:96, 2, :])
                        nc.vector.tensor_copy(qfT[32:64, 3, :], qfT[96:128, 2, :])
                    else:
                        kvcm = work.tile([KVS, BLK], bf16, tag="kvcm", bufs=2)
                        nc.scalar.copy(kvcm[:], pp[:KVS, :])
                        nc.sync.dma_start(ag_in[b][qc][:], kvcm[:])
                        nc.gpsimd.collective_compute(
                            "AllGather",
                            mybir.AluOpType.bypass,
                            replica_groups=RG,
                            ins=[ag_in[b][qc].opt()],
                            outs=[ag_out[b][qc].opt()],
                        )
                return qfT

            def stage_a2(b, qc, knope, vnat, krope):
                """post-AG: rms norm, latent normalize, kv up-projection, k rope."""
                tok = slice(qc * BLK, (qc + 1) * BLK)
                latent = work.tile([128, 4, BLK], bf16, tag="latent", bufs=2)
                nc.sync.dma_start(
                    latent[:],
                    ag_out[b][qc][:KVR, :].rearrange("(kc p) t -> p kc t", p=128),
                )
                kraw = work.tile([ROPE, BLK], bf16, tag="kraw", bufs=2)
                nc.sync.dma_start(kraw[:], ag_out[b][qc][KVR:, :])

                # sum of squares over latent dims (ACT square + PE ones-mm)
                ssq = ps.tile([1, BLK], f32, tag="stats", bufs=1, name="ssq")
                for i in range(4):
                    sqc = work.tile([128, BLK], bf16, tag="sqc", bufs=2)
                    nc.scalar.activation(sqc[:], latent[:, i, :], SQUARE)
                    nc.tensor.matmul(
                        ssq[:], ones[:], sqc[:], start=(i == 0), stop=(i == 3)
                    )
                # rms scale: 1/sqrt(ssq/512+eps) = exp(-0.5*ln(.))
                lnrow = work.tile([1, BLK], f32, tag="lnrow", bufs=2)
                nc.scalar.activation(lnrow[:], ssq[:], LN, bias=eps[:], scale=1.0 / KVR)
                invrow = work.tile([1, BLK], bf16, tag="invrow", bufs=2)
                nc.scalar.activation(invrow[:], lnrow[:], EXP, scale=-0.5)
                invb = work.tile([128, BLK], bf16, tag="invb", bufs=2)
                nc.gpsimd.partition_broadcast(invb[:], invrow[:])

                for i in range(4):
                    nc.vector.tensor_mul(
                        out=latent[:, i, :], in0=latent[:, i, :], in1=invb[:]
                    )

                # k rope from gathered raw rows
                rot = work.tile([128, BLK], bf16, tag="rot", bufs=2)
                kr = krope[:, tok]
                nc.vector.tensor_scalar_mul(rot[0:32, :], kraw[32:64, :], -1.0)
                nc.vector.tensor_copy(rot[32:64, :], kraw[0:32, :])
                nc.vector.tensor_mul(out=kr, in0=kraw[:], in1=cos_sb[0:64, tok])
                nc.vector.tensor_mul(
                    out=rot[0:64, :], in0=rot[0:64, :], in1=sin_sb[0:64, tok]
                )
                nc.vector.tensor_add(out=kr, in0=kr, in1=rot[0:64, :])

                # kv up-projection; v transposed to natural layout
                for mc in range(4):  # [h0 nope, h0 v, h1 nope, h1 v]
                    h = mc // 2
                    pkv = ps.tile([128, BLK], f32, tag="proj", bufs=2, name="pkv")
                    for kc in range(4):
                        nc.tensor.matmul(
                            pkv[:],
                            wb_sb[:, kc, mc * 128 : (mc + 1) * 128],
                            latent[:, kc, :],
                            start=(kc == 0),
                            stop=(kc == 3),
                        )
                    if mc % 2 == 0:
                        nc.scalar.copy(knope[h][:, tok], pkv[:])
                    else:
                        vT = work.tile([128, BLK], bf16, tag="vT", bufs=2)
                        nc.scalar.copy(vT[:], pkv[:])
                        pvt = ps.tile([128, BLK], bf16, tag="pvt", bufs=1, name="pvt")
                        for tt in range(4):
                            nc.tensor.transpose(
                                pvt[:, tt * 128 : (tt + 1) * 128],
                                vT[:, tt * 128 : (tt + 1) * 128],
                                ident[:],
                            )
                        nc.scalar.copy(vnat[h][:, qc * 4 : qc * 4 + 4, :], pvt[:])

            def stage_b(b, qc, qfT, knope, vnat, krope):
                """causal attention for one q-chunk, heads interleaved,
                one-kt software pipeline."""
                n_kt = 4 * (qc + 1)
                yaccs = [
                    ps.tile([VD, BLK], f32, tag=f"yacc{h}", bufs=1, name=f"yacc{h}")
                    for h in range(2)
                ]
                accs = [None, None]
                ests = {}
                for kt in range(n_kt + 1):
                    if kt < n_kt:
                        ks = slice(kt * 128, (kt + 1) * 128)
                        for h in range(2):
                            st = ps.tile(
                                [128, BLK], f32, tag=f"st{h}", bufs=1, name=f"st{h}"
                            )
                            nc.tensor.matmul(
                                st[:],
                                knope[h][:, ks],
                                qfT[:, h, :],
                                start=True,
                                stop=False,
                            )
                            nc.tensor.matmul(
                                st[:],
                                krope[:, ks],
                                qfT[0:64, 2 + h, :],
                                start=False,
                                stop=True,
                            )
                            est = work.tile([128, BLK], bf16, tag="est", bufs=5)
                            nc.scalar.activation(est[:], st[:], EXP)
                            if (kt + 1) * 128 > qc * BLK:
                                nc.gpsimd.affine_select(
                                    out=est[:],
                                    in_=est[:],
                                    compare_op=mybir.AluOpType.is_ge,
                                    fill=0.0,
                                    base=qc * BLK - kt * 128,
                                    pattern=[[1, BLK]],
                                    channel_multiplier=-1,
                                )
                            ests[(h, kt)] = est
                    if kt >= 1:
                        kp = kt - 1
                        for h in range(2):
                            e = ests.pop((h, kp))
                            nc.tensor.matmul(
                                yaccs[h][:],
                                vnat[h][:, kp, :],
                                e[:],
                                start=(kp == 0),
                                stop=(kp == n_kt - 1),
                            )
                            if kp == 0:
                                acc = work.tile(
                                    [128, BLK], f32, tag=f"acc{h}", bufs=2
                                )
                                accs[h] = acc
                                nc.vector.tensor_copy(acc[:], e[:])
                            else:
                                nc.vector.tensor_add(
                                    out=accs[h][:], in0=accs[h][:], in1=e[:]
                                )

                for h in range(2):
                    accsum = work.tile([128, BLK], f32, tag="accsum", bufs=1)
                    nc.gpsimd.partition_all_reduce(
                        accsum[:], accs[h][:], channels=128,
                        reduce_op=bass_isa.ReduceOp.add,
                    )
                    rec = work.tile([128, BLK], f32, tag="rec", bufs=2)
                    nc.vector.reciprocal(rec[:], accsum[:])
                    ysb = work.tile([VD, BLK], bf16, tag="ysb", bufs=2)
                    nc.vector.tensor_mul(out=ysb[:], in0=yaccs[h][:], in1=rec[:])
                    for jj in range(2):
                        nc.sync.dma_start(
                            y_in[b][qc * 2 + jj, h * VD : (h + 1) * VD, :],
                            ysb[:, jj * 256 : (jj + 1) * 256],
                        )

            def emit_a2a(b):
                nc.gpsimd.collective_compute(
                    "AllToAll",
                    mybir.AluOpType.bypass,
                    replica_groups=RG,
                    ins=[y_in[b].opt()],
                    outs=[y_out[b].opt()],
                )

            def emit_wo(b):
                """wo projection for this batch's gathered token slice."""
                a2a = wop.tile([128, 16, 256], bf16, tag="a2a", bufs=1, name="a2a")
                for kc in range(16):
                    nc.sync.dma_start(
                        a2a[:, kc, :],
                        y_out[b][kc // 2, (kc % 2) * 128 : (kc % 2) * 128 + 128, :],
                    )
                for n in range(4):
                    wts = []
                    for half in range(2):
                        wt = wop.tile([128, 8, 512], bf16, tag="wt", bufs=2)
                        nc.sync.dma_start(
                            wt[:],
                            wo_d[
                                half * 1024 : (half + 1) * 1024,
                                n * 512 : (n + 1) * 512,
                            ].rearrange("(kc p) c -> p kc c", p=128),
                        )
                        wts.append(wt)
                    pouts = [
                        ps.tile([128, 512], f32, tag="proj", bufs=2, name="outp")
                        for _ in range(2)
                    ]
                    for kc in range(16):
                        for tt in range(2):
                            nc.tensor.matmul(
                                pouts[tt][:],
                                a2a[:, kc, tt * 128 : (tt + 1) * 128],
                                wts[kc // 8][:, kc % 8, :],
                                start=(kc == 0),
                                stop=(kc == 15),
                            )
                    for tt in range(2):
                        osb = wop.tile([128, 512], f32, tag="osb", bufs=2)
                        nc.scalar.copy(osb[:], pouts[tt][:])
                        nc.sync.dma_start(
                            out_d[
                                b, tt * 128 : (tt + 1) * 128, n * 512 : (n + 1) * 512
                            ],
                            osb[:],
                        )

            # ---- software-pipelined schedule ----
            pending_wo = None
            for b in range(B):
                knope, vnat, krope = alloc_kv()
                st_a = {}
                st_a[0] = stage_a1(b, 0)
                st_a[1] = stage_a1(b, 1)
                if pending_wo is not None:
                    emit_wo(pending_wo)
                    pending_wo = None
                stage_a2(b, 0, knope, vnat, krope)
                st_a[2] = stage_a1(b, 2)
                stage_a2(b, 1, knope, vnat, krope)
                stage_b(b, 0, st_a[0], knope, vnat, krope)
                st_a[3] = stage_a1(b, 3)
                stage_a2(b, 2, knope, vnat, krope)
                stage_b(b, 1, st_a[1], knope, vnat, krope)
                stage_a2(b, 3, knope, vnat, krope)
                stage_b(b, 2, st_a[2], knope, vnat, krope)
                stage_b(b, 3, st_a[3], knope, vnat, krope)
                emit_a2a(b)
                pending_wo = b
            emit_wo(pending_wo)

    nc.compile()
    return nc


def host_prep(x, wq, wkv_a, wkv_b, wo, kv_norm_w):
    scale = np.float32(QKD ** -0.5)
    inv = (1.0 / (10000.0 ** (np.arange(0, ROPE, 2, dtype=np.float32) / ROPE))).astype(
        np.float32
    )
    f = np.outer(np.arange(T, dtype=np.float32), inv)
    cos32 = np.cos(f).T.astype(np.float32)
    sin32 = np.sin(f).T.astype(np.float32)
    cos128 = np.ascontiguousarray(np.concatenate([cos32] * 4, 0)).astype(BF)
    sin128 = np.ascontiguousarray(np.concatenate([sin32] * 4, 0)).astype(BF)
    wkv_bw = (wkv_b * kv_norm_w[:, None]).astype(np.float32)
    xt = np.ascontiguousarray(x.reshape(B * T, D).T).astype(BF)
    wo_c = np.ascontiguousarray(wo).astype(BF)
    wq_r = wq.reshape(D, H, QKD)

    in_maps = []
    for c in range(NCORES):
        h0 = HPC * c
        w1 = np.concatenate(
            [
                wq_r[:, h0, :NOPE] * scale,
                wq_r[:, h0 + 1, :NOPE] * scale,
                wq_r[:, h0, NOPE:] * scale,
                wq_r[:, h0 + 1, NOPE:] * scale,
                wkv_a[:, c * KVS : (c + 1) * KVS],
            ],
            axis=1,
        ).astype(BF)
        wb = np.ascontiguousarray(
            wkv_bw[:, h0 * (NOPE + VD) : (h0 + 2) * (NOPE + VD)]
        ).astype(BF)
        in_maps.append(
            {
                "xt": xt,
                "w1": np.ascontiguousarray(w1),
                "wb": wb,
                "wo": wo_c,
                "cos": cos128,
                "sin": sin128,
            }
        )
    return in_maps


_NC = None


def kernel(x, wq, wkv_a, wkv_b, wo, kv_norm_w, _trace=False):
    global _NC
    if _NC is None:
        _NC = build_program()
    in_maps = host_prep(
        np.asarray(x, np.float32),
        np.asarray(wq, np.float32),
        np.asarray(wkv_a, np.float32),
        np.asarray(wkv_b, np.float32),
        np.asarray(wo, np.float32),
        np.asarray(kv_norm_w, np.float32),
    )
    res = run_bass_kernel_spmd(_NC, in_maps, list(range(NCORES)), trace=_trace)
    out = np.empty((B, T, D), np.float32)
    cw = T // NCORES
    for c in range(NCORES):
        oc = res.results[c]["out"]  # (B, 256, D)
        for b in range(B):
            out[b, c * cw : (c + 1) * cw, :] = oc[b]
    kernel.last_results = res
    return out


# revision 10
# speedup vs baseline: 2.2776x; 1.0838x over previous
"""Multi-Head Latent Attention (MLA) forward on 8 Trainium2 NeuronCores.

Sharding: tensor-parallel over heads (16 heads -> 2 per core). V2 design:
  - x is transposed to feature-major layout on the HOST (pure layout prep),
    so the device streams xT tiles straight into the projection matmuls --
    no PE transposes, no PSUM round-trips for x.
  - all matmul operands are bf16 (fp32 PSUM accumulation); softmax stats and
    normalization factors stay fp32 on the scalar engine.
  - each core computes q projections for its 2 heads plus a 1/8 column slice
    of the latent-kv encoding; per-block AllGather (Shared output buffers)
    assembles the full latent, hidden behind compute.
  - RMS norm: sum-of-squares via ACT square + PE ones-matmul; the latent is
    normalized BEFORE the up-projection (norm weight folded into wkv_b).
  - causal attention in transposed-score layout St[k, q], softmax without
    max subtraction (scores are O(1)); the two heads are interleaved with a
    one-kt software pipeline so the PE never waits on exp; the softmax
    denominator is accumulated with vector adds (fp32) and reduced across
    partitions with gpsimd partition_all_reduce -- no serial gpsimd chains.
  - per-batch AllToAll exchanges head outputs; each core computes a
    256-token slice of the wo projection. Batch 0's wo is emitted after
    batch 1's first projection stages so the PE keeps streaming through the
    collective latency.
Output slices are disjoint; the host just concatenates them.
"""
import sys

if "/opt/trn_rl_repo" not in sys.path:
    sys.path.insert(0, "/opt/trn_rl_repo")

import numpy as np
import ml_dtypes
import concourse.bacc as bacc
import concourse.mybir as mybir
from concourse import tile
from concourse import bass_isa
from concourse.masks import make_identity
from concourse.bass_utils import run_bass_kernel_spmd

H, NOPE, ROPE, VD, KVR, QKD = 16, 128, 64, 128, 512, 192
B, T, D = 2, 2048, 2048
NCORES, HPC, BLK = 8, 2, 512
KVC = KVR + ROPE  # 576 latent+rope columns
KVS = KVC // NCORES  # 72-column slice per core
W1N = HPC * QKD + KVS  # 456 projection columns per core
f32 = mybir.dt.float32
bf16 = mybir.dt.bfloat16
EXP = mybir.ActivationFunctionType.Exp
LN = mybir.ActivationFunctionType.Ln
SQUARE = mybir.ActivationFunctionType.Square
BF = ml_dtypes.bfloat16


def _patch_act_tables():
    """Make the act-table-load pass serve Exp/Ln/Square from the one set that
    contains them all (natural_log_exp_and_others), so interleaved activations
    don't thrash table loads."""
    import concourse.bacc as _bacc

    orig = _bacc.get_activation_tables
    if getattr(_bacc, "_mla_act_patch", False):
        return
    _bacc._mla_act_patch = True

    def patched(arch):
        d = dict(orig(arch))
        if "natural_log_exp_and_others" in d:
            for name in ("exp_and_others", "natural_log", "exp_and_friends"):
                if name in d:
                    d[name] = set()
        return d

    _bacc.get_activation_tables = patched


def build_program():
    _patch_act_tables()
    nc = bacc.Bacc("TRN2", target_bir_lowering=False, debug=False, num_devices=NCORES)
    xt_d = nc.dram_tensor("xt", [D, B * T], bf16, kind="ExternalInput")
    w1_d = nc.dram_tensor("w1", [D, W1N], bf16, kind="ExternalInput")
    wb_d = nc.dram_tensor("wb", [KVR, HPC * (NOPE + VD)], bf16, kind="ExternalInput")
    wo_d = nc.dram_tensor("wo", [H * VD, D], bf16, kind="ExternalInput")
    cos_d = nc.dram_tensor("cos", [128, T], bf16, kind="ExternalInput")
    sin_d = nc.dram_tensor("sin", [128, T], bf16, kind="ExternalInput")
    out_d = nc.dram_tensor("out", [B, T // NCORES, D], f32, kind="ExternalOutput")

    RG = [list(range(NCORES))]

    with tile.TileContext(nc) as tc:
        with (
            tc.tile_pool(name="dram", bufs=1, space="DRAM") as dram,
            tc.tile_pool(name="const", bufs=1) as const,
            tc.tile_pool(name="wpool", bufs=1) as wpool,
            tc.tile_pool(name="kvpool", bufs=1) as kvpool,
            tc.tile_pool(name="xpool", bufs=1) as xpool,
            tc.tile_pool(name="work", bufs=1) as work,
            tc.tile_pool(name="wop", bufs=1) as wop,
            tc.tile_pool(name="ps", bufs=1, space="PSUM") as ps,
        ):
            y_in = [
                dram.tile([NCORES, HPC * VD, 256], bf16, name=f"y_in{b}")
                for b in range(B)
            ]
            y_out = [
                dram.tile([NCORES, HPC * VD, 256], bf16, name=f"y_out{b}")
                for b in range(B)
            ]
            ag_in = [
                [dram.tile([KVS, BLK], bf16, name=f"ag_in{b}_{q}") for q in range(4)]
                for b in range(B)
            ]
            ag_out = [
                [
                    dram.tile(
                        [KVC, BLK], bf16, name=f"ag_out{b}_{q}", addr_space="Shared"
                    )
                    for q in range(4)
                ]
                for b in range(B)
            ]

            ident_f = const.tile([128, 128], f32, tag="ident_f")
            make_identity(nc, ident_f)
            ident = const.tile([128, 128], bf16, tag="ident")
            nc.vector.tensor_copy(ident[:], ident_f[:])
            ones_f = const.tile([128, 1], f32, tag="ones_f")
            nc.gpsimd.memset(ones_f[:], 1.0)
            ones = const.tile([128, 1], bf16, tag="ones")
            nc.vector.tensor_copy(ones[:], ones_f[:])
            eps = const.tile([1, 1], f32, tag="eps")
            nc.gpsimd.memset(eps[:], 1e-6)
            stats = ps.tile([65, BLK], f32, tag="stats", bufs=1, name="stats")
            w1_sb = wpool.tile([128, 16, W1N], bf16, tag="w1")
            nc.sync.dma_start(w1_sb[:], w1_d[:].rearrange("(kc p) m -> p kc m", p=128))
            wb_sb = wpool.tile([128, 4, 512], bf16, tag="wb")
            nc.sync.dma_start(wb_sb[:], wb_d[:].rearrange("(kc p) m -> p kc m", p=128))
            cos_sb = wpool.tile([128, T], bf16, tag="cos")
            nc.sync.dma_start(cos_sb[:], cos_d[:])
            sin_sb = wpool.tile([128, T], bf16, tag="sin")
            nc.sync.dma_start(sin_sb[:], sin_d[:])

            # per-batch kv staging (bufs=2 -> batches rotate slots, no WAR stall)
            def alloc_kv():
                knope = [
                    kvpool.tile(
                        [NOPE, T], bf16, tag=f"knope{h}", bufs=2, name=f"knope{h}"
                    )
                    for h in range(2)
                ]
                vnat = [
                    kvpool.tile(
                        [128, 16, VD], bf16, tag=f"vnat{h}", bufs=2, name=f"vnat{h}"
                    )
                    for h in range(2)
                ]
                krope = kvpool.tile([ROPE, T], bf16, tag="krope", bufs=2)
                return knope, vnat, krope

            def stage_a1(b, qc):
                """xT-tile load + q/kvc-slice projection + q rope + AllGather."""
                tok = slice(qc * BLK, (qc + 1) * BLK)
                col0 = b * T + qc * BLK
                xTc = xpool.tile([128, 16, BLK], bf16, tag="xTc", bufs=2)
                nc.sync.dma_start(
                    xTc[:],
                    xt_d[:, col0 : col0 + BLK].rearrange("(kc p) t -> p kc t", p=128),
                )
                qfT = work.tile([128, 4, BLK], bf16, tag="qfT", bufs=4)
                csb = cos_sb[:, tok]
                ssb = sin_sb[:, tok]

                for mc in (3, 0, 1, 2):
                    m0 = mc * 128
                    m1 = min(m0 + 128, W1N)
                    pp = ps.tile([128, BLK], f32, tag="proj", bufs=2, name="projp")
                    for kc in range(16):
                        nc.tensor.matmul(
                            pp[: m1 - m0, :],
                            w1_sb[:, kc, m0:m1],
                            xTc[:, kc, :],
                            start=(kc == 0),
                            stop=(kc == 15),
                        )
                    # post-process each chunk right away to free the PSUM slot
                    if mc < 2:
                        nc.scalar.copy(qfT[:, mc, :], pp[:])
                    elif mc == 2:
                        rot = work.tile([128, BLK], bf16, tag="rot", bufs=2)
                        for hh in range(2):
                            r0 = hh * 64
                            nc.vector.tensor_scalar_mul(
                                rot[r0 : r0 + 32, :], pp[r0 + 32 : r0 + 64, :], -1.0
                            )
                            nc.vector.tensor_copy(
                                rot[r0 + 32 : r0 + 64, :], pp[r0 : r0 + 32, :]
                            )
                        nc.vector.tensor_mul(out=qfT[:, 2, :], in0=pp[:], in1=csb)
                        nc.vector.tensor_mul(out=rot[:], in0=rot[:], in1=ssb)
                        nc.vector.tensor_add(
                            out=qfT[:, 2, :], in0=qfT[:, 2, :], in1=rot[:]
                        )
                        # h1 roped rows 64:128 -> chunk 3 rows 0:64
                        nc.vector.tensor_copy(qfT[0:32, 3, :], qfT[64:96, 2, :])
                        nc.vector.tensor_copy(qfT[32:64, 3, :], qfT[96:128, 2, :])
                    else:
                        kvcm = work.tile([KVS, BLK], bf16, tag="kvcm", bufs=2)
                        nc.scalar.copy(kvcm[:], pp[:KVS, :])
                        nc.sync.dma_start(ag_in[b][qc][:], kvcm[:])
                        nc.gpsimd.collective_compute(
                            "AllGather",
                            mybir.AluOpType.bypass,
                            replica_groups=RG,
                            ins=[ag_in[b][qc].opt()],
                            outs=[ag_out[b][qc].opt()],
                        )
                return qfT

            def stage_a2(b, qc, knope, vnat, krope):
                """post-AG: rms norm, latent normalize, kv up-projection, k rope."""
                tok = slice(qc * BLK, (qc + 1) * BLK)
                latent = work.tile([128, 4, BLK], bf16, tag="latent", bufs=2)
                nc.sync.dma_start(
                    latent[:],
                    ag_out[b][qc][:KVR, :].rearrange("(kc p) t -> p kc t", p=128),
                )
                kraw = work.tile([ROPE, BLK], bf16, tag="kraw", bufs=2)
                nc.sync.dma_start(kraw[:], ag_out[b][qc][KVR:, :])

                # sum of squares over latent dims (ACT square + PE ones-mm)
                ssq = stats[0:1, :]
                for i in range(4):
                    sqc = work.tile([128, BLK], bf16, tag="sqc", bufs=1)
                    nc.scalar.activation(sqc[:], latent[:, i, :], SQUARE)
                    nc.tensor.matmul(
                        ssq[:], ones[:], sqc[:], start=(i == 0), stop=(i == 3)
                    )
                # rms scale: 1/sqrt(ssq/512+eps) = exp(-0.5*ln(.))
                lnrow = work.tile([1, BLK], f32, tag="lnrow", bufs=2)
                nc.scalar.activation(lnrow[:], ssq[:], LN, bias=eps[:], scale=1.0 / KVR)
                invrow = work.tile([1, BLK], bf16, tag="invrow", bufs=2)
                nc.scalar.activation(invrow[:], lnrow[:], EXP, scale=-0.5)
                invb = work.tile([128, BLK], bf16, tag="invb", bufs=1)
                nc.gpsimd.partition_broadcast(invb[:], invrow[:])

                for i in range(4):
                    nc.vector.tensor_mul(
                        out=latent[:, i, :], in0=latent[:, i, :], in1=invb[:]
                    )

                # k rope from gathered raw rows
                rot = work.tile([128, BLK], bf16, tag="rot", bufs=2)
                kr = krope[:, tok]
                nc.vector.tensor_scalar_mul(rot[0:32, :], kraw[32:64, :], -1.0)
                nc.vector.tensor_copy(rot[32:64, :], kraw[0:32, :])
                nc.vector.tensor_mul(out=kr, in0=kraw[:], in1=cos_sb[0:64, tok])
                nc.vector.tensor_mul(
                    out=rot[0:64, :], in0=rot[0:64, :], in1=sin_sb[0:64, tok]
                )
                nc.vector.tensor_add(out=kr, in0=kr, in1=rot[0:64, :])

                # kv up-projection; v transposed to natural layout
                for mc in range(4):  # [h0 nope, h0 v, h1 nope, h1 v]
                    h = mc // 2
                    pkv = ps.tile([128, BLK], f32, tag="proj", bufs=2, name="pkv")
                    for kc in range(4):
                        nc.tensor.matmul(
                            pkv[:],
                            wb_sb[:, kc, mc * 128 : (mc + 1) * 128],
                            latent[:, kc, :],
                            start=(kc == 0),
                            stop=(kc == 3),
                        )
                    if mc % 2 == 0:
                        nc.scalar.copy(knope[h][:, tok], pkv[:])
                    else:
                        vT = work.tile([128, BLK], bf16, tag="vT", bufs=2)
                        nc.scalar.copy(vT[:], pkv[:])
                        pvt = ps.tile([128, BLK], bf16, tag="pvt", bufs=1, name="pvt")
                        for tt in range(4):
                            nc.tensor.transpose(
                                pvt[:, tt * 128 : (tt + 1) * 128],
                                vT[:, tt * 128 : (tt + 1) * 128],
                                ident[:],
                            )
                        nc.scalar.copy(vnat[h][:, qc * 4 : qc * 4 + 4, :], pvt[:])

            def stage_b(b, qc, qfT, knope, vnat, krope):
                """causal attention for one q-chunk, heads interleaved,
                one-kt software pipeline."""
                n_kt = 4 * (qc + 1)
                yaccs = [
                    ps.tile([VD, BLK], f32, tag=f"yacc{h}", bufs=1, name=f"yacc{h}")
                    for h in range(2)
                ]
                accs = [None, None]
                ests = {}
                for kt in range(n_kt + 1):
                    if kt < n_kt:
                        ks = slice(kt * 128, (kt + 1) * 128)
                        for h in range(2):
                            st = ps.tile(
                                [128, BLK], f32, tag=f"st{h}", bufs=1, name=f"st{h}"
                            )
                            nc.tensor.matmul(
                                st[:],
                                knope[h][:, ks],
                                qfT[:, h, :],
                                start=True,
                                stop=False,
                            )
                            nc.tensor.matmul(
                                st[:],
                                krope[:, ks],
                                qfT[0:64, 2 + h, :],
                                start=False,
                                stop=True,
                            )
                            est = work.tile([128, BLK], bf16, tag="est", bufs=4)
                            nc.scalar.activation(est[:], st[:], EXP)
                            if (kt + 1) * 128 > qc * BLK:
                                nc.gpsimd.affine_select(
                                    out=est[:],
                                    in_=est[:],
                                    compare_op=mybir.AluOpType.is_ge,
                                    fill=0.0,
                                    base=qc * BLK - kt * 128,
                                    pattern=[[1, BLK]],
                                    channel_multiplier=-1,
                                )
                            ests[(h, kt)] = est
                    if kt >= 1:
                        kp = kt - 1
                        for h in range(2):
                            e = ests.pop((h, kp))
                            nc.tensor.matmul(
                                yaccs[h][:],
                                vnat[h][:, kp, :],
                                e[:],
                                start=(kp == 0),
                                stop=(kp == n_kt - 1),
                            )
                            if kp == 0:
                                acc = work.tile(
                                    [128, BLK], f32, tag=f"acc{h}", bufs=2
                                )
                                accs[h] = acc
                                nc.vector.tensor_copy(acc[:], e[:])
                            else:
                                nc.vector.tensor_add(
                                    out=accs[h][:], in0=accs[h][:], in1=e[:]
                                )

                for h in range(2):
                    srow = stats[32 + 32 * h : 33 + 32 * h, :]
                    accb = work.tile([128, BLK], bf16, tag="accb", bufs=2)
                    nc.vector.tensor_copy(accb[:], accs[h][:])
                    nc.tensor.matmul(
                        srow,
                        ones[:],
                        accb[:],
                        start=True,
                        stop=True,
                    )
                    recro = work.tile([1, BLK], f32, tag="recro", bufs=2)
                    nc.vector.reciprocal(recro[:], srow)
                    recb16 = work.tile([1, BLK], bf16, tag="recb16", bufs=2)
                    nc.vector.tensor_copy(recb16[:], recro[:])
                    recb = work.tile([128, BLK], bf16, tag="recb", bufs=2)
                    nc.gpsimd.partition_broadcast(recb[:], recb16[:])
                    ysb = work.tile([VD, BLK], bf16, tag="ysb", bufs=2)
                    nc.vector.tensor_mul(out=ysb[:], in0=yaccs[h][:], in1=recb[:])
                    for jj in range(2):
                        nc.sync.dma_start(
                            y_in[b][qc * 2 + jj, h * VD : (h + 1) * VD, :],
                            ysb[:, jj * 256 : (jj + 1) * 256],
                        )

            def emit_a2a(b):
                nc.gpsimd.collective_compute(
                    "AllToAll",
                    mybir.AluOpType.bypass,
                    replica_groups=RG,
                    ins=[y_in[b].opt()],
                    outs=[y_out[b].opt()],
                )

            def emit_wo(b):
                """wo projection for this batch's gathered token slice.
                wt streams are issued before the a2a-gated gather so the
                weight transfers complete during the collective."""

                def load_wt(n):
                    pair = []
                    for half in range(2):
                        wt = wop.tile([128, 8, 512], bf16, tag="wt", bufs=4, name="wt")
                        nc.sync.dma_start(
                            wt[:],
                            wo_d[
                                half * 1024 : (half + 1) * 1024,
                                n * 512 : (n + 1) * 512,
                            ].rearrange("(kc p) c -> p kc c", p=128),
                        )
                        pair.append(wt)
                    return pair

                wts = {0: load_wt(0), 1: load_wt(1)}
                a2a = wop.tile([128, 16, 256], bf16, tag="a2a", bufs=1, name="a2a")
                nc.sync.dma_start(
                    a2a[:],
                    y_out[b][:].rearrange("c (two p) t -> p (c two) t", p=128),
                )
                for n in range(4):
                    if n + 2 < 4:
                        wts[n + 2] = load_wt(n + 2)
                    pouts = [
                        ps.tile([128, 512], f32, tag="proj", bufs=2, name="outp")
                        for _ in range(2)
                    ]
                    for kc in range(16):
                        for tt in range(2):
                            nc.tensor.matmul(
                                pouts[tt][:],
                                a2a[:, kc, tt * 128 : (tt + 1) * 128],
                                wts[n][kc // 8][:, kc % 8, :],
                                start=(kc == 0),
                                stop=(kc == 15),
                            )
                    for tt in range(2):
                        osb = wop.tile([128, 512], f32, tag="osb", bufs=2)
                        nc.scalar.copy(osb[:], pouts[tt][:])
                        nc.sync.dma_start(
                            out_d[
                                b, tt * 128 : (tt + 1) * 128, n * 512 : (n + 1) * 512
                            ],
                            osb[:],
                        )

            # ---- software-pipelined schedule ----
            pending_wo = None
            for b in range(B):
                knope, vnat, krope = alloc_kv()
                st_a = {}
                st_a[0] = stage_a1(b, 0)
                st_a[1] = stage_a1(b, 1)
                st_a[2] = stage_a1(b, 2)
                stage_a2(b, 0, knope, vnat, krope)
                st_a[3] = stage_a1(b, 3)
                stage_a2(b, 1, knope, vnat, krope)
                if pending_wo is not None:
                    emit_wo(pending_wo)
                    pending_wo = None
                stage_b(b, 0, st_a[0], knope, vnat, krope)
                stage_a2(b, 2, knope, vnat, krope)
                stage_b(b, 1, st_a[1], knope, vnat, krope)
                stage_a2(b, 3, knope, vnat, krope)
                stage_b(b, 2, st_a[2], knope, vnat, krope)
                stage_b(b, 3, st_a[3], knope, vnat, krope)
                emit_a2a(b)
                pending_wo = b
            emit_wo(pending_wo)

    nc.compile()
    return nc


def host_prep(x, wq, wkv_a, wkv_b, wo, kv_norm_w):
    scale = np.float32(QKD ** -0.5)
    inv = (1.0 / (10000.0 ** (np.arange(0, ROPE, 2, dtype=np.float32) / ROPE))).astype(
        np.float32
    )
    f = np.outer(np.arange(T, dtype=np.float32), inv)
    cos32 = np.cos(f).T.astype(np.float32)
    sin32 = np.sin(f).T.astype(np.float32)
    cos128 = np.ascontiguousarray(np.concatenate([cos32] * 4, 0)).astype(BF)
    sin128 = np.ascontiguousarray(np.concatenate([sin32] * 4, 0)).astype(BF)
    wkv_bw = (wkv_b * kv_norm_w[:, None]).astype(np.float32)
    xt = np.ascontiguousarray(x.reshape(B * T, D).T).astype(BF)
    wo_c = np.ascontiguousarray(wo).astype(BF)
    wq_r = wq.reshape(D, H, QKD)

    in_maps = []
    for c in range(NCORES):
        h0 = HPC * c
        w1 = np.concatenate(
            [
                wq_r[:, h0, :NOPE] * scale,
                wq_r[:, h0 + 1, :NOPE] * scale,
                wq_r[:, h0, NOPE:] * scale,
                wq_r[:, h0 + 1, NOPE:] * scale,
                wkv_a[:, c * KVS : (c + 1) * KVS],
            ],
            axis=1,
        ).astype(BF)
        wb = np.ascontiguousarray(
            wkv_bw[:, h0 * (NOPE + VD) : (h0 + 2) * (NOPE + VD)]
        ).astype(BF)
        in_maps.append(
            {
                "xt": xt,
                "w1": np.ascontiguousarray(w1),
                "wb": wb,
                "wo": wo_c,
                "cos": cos128,
                "sin": sin128,
            }
        )
    return in_maps


_NC = None


def kernel(x, wq, wkv_a, wkv_b, wo, kv_norm_w, _trace=False):
    global _NC
    if _NC is None:
        _NC = build_program()
    in_maps = host_prep(
        np.asarray(x, np.float32),
        np.asarray(wq, np.float32),
        np.asarray(wkv_a, np.float32),
        np.asarray(wkv_b, np.float32),
        np.asarray(wo, np.float32),
        np.asarray(kv_norm_w, np.float32),
    )
    res = run_bass_kernel_spmd(_NC, in_maps, list(range(NCORES)), trace=_trace)
    out = np.empty((B, T, D), np.float32)
    cw = T // NCORES
    for c in range(NCORES):
        oc = res.results[c]["out"]  # (B, 256, D)
        for b in range(B):
            out[b, c * cw : (c + 1) * cw, :] = oc[b]
    kernel.last_results = res
    return out


# revision 11
# speedup vs baseline: 2.4614x; 1.0807x over previous
"""Multi-Head Latent Attention (MLA) forward on 8 Trainium2 NeuronCores.

Sharding: tensor-parallel over heads (16 heads -> 2 per core). V2 design:
  - x is transposed to feature-major layout on the HOST (pure layout prep),
    so the device streams xT tiles straight into the projection matmuls --
    no PE transposes, no PSUM round-trips for x.
  - all matmul operands are bf16 (fp32 PSUM accumulation); softmax stats and
    normalization factors stay fp32 on the scalar engine.
  - each core computes q projections for its 2 heads plus a 1/8 column slice
    of the latent-kv encoding; per-block AllGather (Shared output buffers)
    assembles the full latent, hidden behind compute.
  - RMS norm: sum-of-squares via ACT square + PE ones-matmul; the latent is
    normalized BEFORE the up-projection (norm weight folded into wkv_b).
  - causal attention in transposed-score layout St[k, q], softmax without
    max subtraction (scores are O(1)); the two heads are interleaved with a
    one-kt software pipeline so the PE never waits on exp; the softmax
    denominator is accumulated with vector adds (fp32) and reduced across
    partitions with gpsimd partition_all_reduce -- no serial gpsimd chains.
  - per-batch AllToAll exchanges head outputs; each core computes a
    256-token slice of the wo projection. Batch 0's wo is emitted after
    batch 1's first projection stages so the PE keeps streaming through the
    collective latency.
Output slices are disjoint; the host just concatenates them.
"""
import sys

if "/opt/trn_rl_repo" not in sys.path:
    sys.path.insert(0, "/opt/trn_rl_repo")

import numpy as np
import ml_dtypes
import concourse.bacc as bacc
import concourse.mybir as mybir
from concourse import tile
from concourse import bass_isa
from concourse.masks import make_identity
from concourse.bass_utils import run_bass_kernel_spmd

H, NOPE, ROPE, VD, KVR, QKD = 16, 128, 64, 128, 512, 192
B, T, D = 2, 2048, 2048
NCORES, HPC, BLK = 8, 2, 512
KVC = KVR + ROPE  # 576 latent+rope columns
KVS = KVC // NCORES  # 72-column slice per core
W1N = HPC * QKD + KVS  # 456 projection columns per core
f32 = mybir.dt.float32
bf16 = mybir.dt.bfloat16
EXP = mybir.ActivationFunctionType.Exp
LN = mybir.ActivationFunctionType.Ln
SQUARE = mybir.ActivationFunctionType.Square
BF = ml_dtypes.bfloat16


def _patch_act_tables():
    """Make the act-table-load pass serve Exp/Ln/Square from the one set that
    contains them all (natural_log_exp_and_others), so interleaved activations
    don't thrash table loads."""
    import concourse.bacc as _bacc

    orig = _bacc.get_activation_tables
    if getattr(_bacc, "_mla_act_patch", False):
        return
    _bacc._mla_act_patch = True

    def patched(arch):
        d = dict(orig(arch))
        if "natural_log_exp_and_others" in d:
            for name in ("exp_and_others", "natural_log", "exp_and_friends"):
                if name in d:
                    d[name] = set()
        return d

    _bacc.get_activation_tables = patched


def build_program():
    _patch_act_tables()
    nc = bacc.Bacc("TRN2", target_bir_lowering=False, debug=False, num_devices=NCORES)
    xt_d = nc.dram_tensor("xt", [D, B * T], bf16, kind="ExternalInput")
    w1_d = nc.dram_tensor("w1", [D, W1N], bf16, kind="ExternalInput")
    wb_d = nc.dram_tensor("wb", [KVR, HPC * (NOPE + VD)], bf16, kind="ExternalInput")
    wo_d = nc.dram_tensor("wo", [H * VD, D], bf16, kind="ExternalInput")
    cos_d = nc.dram_tensor("cos", [128, T], bf16, kind="ExternalInput")
    sin_d = nc.dram_tensor("sin", [128, T], bf16, kind="ExternalInput")
    out_d = nc.dram_tensor("out", [B, T // NCORES, D], f32, kind="ExternalOutput")

    RG = [list(range(NCORES))]

    with tile.TileContext(nc) as tc:
        with (
            tc.tile_pool(name="dram", bufs=1, space="DRAM") as dram,
            tc.tile_pool(name="const", bufs=1) as const,
            tc.tile_pool(name="wpool", bufs=1) as wpool,
            tc.tile_pool(name="kvpool", bufs=1) as kvpool,
            tc.tile_pool(name="xpool", bufs=1) as xpool,
            tc.tile_pool(name="work", bufs=1) as work,
            tc.tile_pool(name="wop", bufs=1) as wop,
            tc.tile_pool(name="ps", bufs=1, space="PSUM") as ps,
        ):
            y_in = [
                dram.tile([NCORES, HPC * VD, 256], bf16, name=f"y_in{b}")
                for b in range(B)
            ]
            y_out = [
                dram.tile([NCORES, HPC * VD, 256], bf16, name=f"y_out{b}")
                for b in range(B)
            ]
            ag_in = [
                [dram.tile([KVS, 2 * BLK], bf16, name=f"ag_in{b}_{p}") for p in range(2)]
                for b in range(B)
            ]
            ag_out = [
                [
                    dram.tile(
                        [KVC, 2 * BLK], bf16, name=f"ag_out{b}_{p}", addr_space="Shared"
                    )
                    for p in range(2)
                ]
                for b in range(B)
            ]
            warm_in = dram.tile([1, 64], bf16, name="warm_in")
            warm_out = dram.tile([NCORES, 64], bf16, name="warm_out", addr_space="Shared")
            nc.gpsimd.collective_compute(
                "AllGather",
                mybir.AluOpType.bypass,
                replica_groups=RG,
                ins=[warm_in.opt()],
                outs=[warm_out.opt()],
            )

            ident_f = const.tile([128, 128], f32, tag="ident_f")
            make_identity(nc, ident_f)
            ident = const.tile([128, 128], bf16, tag="ident")
            nc.vector.tensor_copy(ident[:], ident_f[:])
            ones_f = const.tile([128, 1], f32, tag="ones_f")
            nc.gpsimd.memset(ones_f[:], 1.0)
            ones = const.tile([128, 1], bf16, tag="ones")
            nc.vector.tensor_copy(ones[:], ones_f[:])
            eps = const.tile([1, 1], f32, tag="eps")
            nc.gpsimd.memset(eps[:], 1e-6)
            stats = ps.tile([65, BLK], f32, tag="stats", bufs=1, name="stats")
            w1_sb = wpool.tile([128, 16, W1N], bf16, tag="w1")
            nc.sync.dma_start(w1_sb[:], w1_d[:].rearrange("(kc p) m -> p kc m", p=128))
            wb_sb = wpool.tile([128, 4, 512], bf16, tag="wb")
            nc.sync.dma_start(wb_sb[:], wb_d[:].rearrange("(kc p) m -> p kc m", p=128))
            cos_sb = wpool.tile([128, T], bf16, tag="cos")
            nc.sync.dma_start(cos_sb[:], cos_d[:])
            sin_sb = wpool.tile([128, T], bf16, tag="sin")
            nc.sync.dma_start(sin_sb[:], sin_d[:])

            # per-batch kv staging (bufs=2 -> batches rotate slots, no WAR stall)
            def alloc_kv():
                knope = [
                    kvpool.tile(
                        [NOPE, T], bf16, tag=f"knope{h}", bufs=2, name=f"knope{h}"
                    )
                    for h in range(2)
                ]
                vnat = [
                    kvpool.tile(
                        [128, 16, VD], bf16, tag=f"vnat{h}", bufs=2, name=f"vnat{h}"
                    )
                    for h in range(2)
                ]
                krope = kvpool.tile([ROPE, T], bf16, tag="krope", bufs=2)
                return knope, vnat, krope

            def stage_a1(b, qc):
                """xT-tile load + q/kvc-slice projection + q rope + AllGather."""
                tok = slice(qc * BLK, (qc + 1) * BLK)
                col0 = b * T + qc * BLK
                xTc = xpool.tile([128, 16, BLK], bf16, tag="xTc", bufs=2)
                nc.sync.dma_start(
                    xTc[:],
                    xt_d[:, col0 : col0 + BLK].rearrange("(kc p) t -> p kc t", p=128),
                )
                qfT = work.tile([128, 4, BLK], bf16, tag="qfT", bufs=4)
                csb = cos_sb[:, tok]
                ssb = sin_sb[:, tok]

                for mc in (3, 0, 1, 2):
                    m0 = mc * 128
                    m1 = min(m0 + 128, W1N)
                    pp = ps.tile([128, BLK], f32, tag="proj", bufs=2, name="projp")
                    for kc in range(16):
                        nc.tensor.matmul(
                            pp[: m1 - m0, :],
                            w1_sb[:, kc, m0:m1],
                            xTc[:, kc, :],
                            start=(kc == 0),
                            stop=(kc == 15),
                        )
                    # post-process each chunk right away to free the PSUM slot
                    if mc < 2:
                        nc.scalar.copy(qfT[:, mc, :], pp[:])
                    elif mc == 2:
                        rot = work.tile([128, BLK], bf16, tag="rot", bufs=2)
                        for hh in range(2):
                            r0 = hh * 64
                            nc.vector.tensor_scalar_mul(
                                rot[r0 : r0 + 32, :], pp[r0 + 32 : r0 + 64, :], -1.0
                            )
                            nc.vector.tensor_copy(
                                rot[r0 + 32 : r0 + 64, :], pp[r0 : r0 + 32, :]
                            )
                        nc.vector.tensor_mul(out=qfT[:, 2, :], in0=pp[:], in1=csb)
                        nc.vector.tensor_mul(out=rot[:], in0=rot[:], in1=ssb)
                        nc.vector.tensor_add(
                            out=qfT[:, 2, :], in0=qfT[:, 2, :], in1=rot[:]
                        )
                        # h1 roped rows 64:128 -> chunk 3 rows 0:64
                        nc.vector.tensor_copy(qfT[0:32, 3, :], qfT[64:96, 2, :])
                        nc.vector.tensor_copy(qfT[32:64, 3, :], qfT[96:128, 2, :])
                    else:
                        kvcm = work.tile([KVS, BLK], bf16, tag="kvcm", bufs=2)
                        nc.scalar.copy(kvcm[:], pp[:KVS, :])
                        nc.sync.dma_start(
                            ag_in[b][qc // 2][:, (qc % 2) * BLK : (qc % 2 + 1) * BLK],
                            kvcm[:],
                        )
                return qfT

            def emit_ag(b, pair):
                nc.gpsimd.collective_compute(
                    "AllGather",
                    mybir.AluOpType.bypass,
                    replica_groups=RG,
                    ins=[ag_in[b][pair].opt()],
                    outs=[ag_out[b][pair].opt()],
                )

            def stage_a2(b, qc, knope, vnat, krope):
                """post-AG: rms norm, latent normalize, kv up-projection, k rope."""
                tok = slice(qc * BLK, (qc + 1) * BLK)
                half = slice((qc % 2) * BLK, (qc % 2 + 1) * BLK)
                latent = work.tile([128, 4, BLK], bf16, tag="latent", bufs=2)
                nc.sync.dma_start(
                    latent[:],
                    ag_out[b][qc // 2][:KVR, half].rearrange("(kc p) t -> p kc t", p=128),
                )
                kraw = work.tile([ROPE, BLK], bf16, tag="kraw", bufs=2)
                nc.sync.dma_start(kraw[:], ag_out[b][qc // 2][KVR:, half])

                # sum of squares over latent dims (ACT square + PE ones-mm)
                ssq = stats[0:1, :]
                for i in range(4):
                    sqc = work.tile([128, BLK], bf16, tag="sqc", bufs=1)
                    nc.scalar.activation(sqc[:], latent[:, i, :], SQUARE)
                    nc.tensor.matmul(
                        ssq[:], ones[:], sqc[:], start=(i == 0), stop=(i == 3)
                    )
                # rms scale: 1/sqrt(ssq/512+eps) = exp(-0.5*ln(.))
                lnrow = work.tile([1, BLK], f32, tag="lnrow", bufs=2)
                nc.scalar.activation(lnrow[:], ssq[:], LN, bias=eps[:], scale=1.0 / KVR)
                invrow = work.tile([1, BLK], bf16, tag="invrow", bufs=2)
                nc.scalar.activation(invrow[:], lnrow[:], EXP, scale=-0.5)
                invb = work.tile([128, BLK], bf16, tag="invb", bufs=1)
                nc.gpsimd.partition_broadcast(invb[:], invrow[:])

                for i in range(4):
                    nc.vector.tensor_mul(
                        out=latent[:, i, :], in0=latent[:, i, :], in1=invb[:]
                    )

                # k rope from gathered raw rows
                rot = work.tile([128, BLK], bf16, tag="rot", bufs=2)
                kr = krope[:, tok]
                nc.vector.tensor_scalar_mul(rot[0:32, :], kraw[32:64, :], -1.0)
                nc.vector.tensor_copy(rot[32:64, :], kraw[0:32, :])
                nc.vector.tensor_mul(out=kr, in0=kraw[:], in1=cos_sb[0:64, tok])
                nc.vector.tensor_mul(
                    out=rot[0:64, :], in0=rot[0:64, :], in1=sin_sb[0:64, tok]
                )
                nc.vector.tensor_add(out=kr, in0=kr, in1=rot[0:64, :])

                # kv up-projection; v transposed to natural layout
                for mc in range(4):  # [h0 nope, h0 v, h1 nope, h1 v]
                    h = mc // 2
                    pkv = ps.tile([128, BLK], f32, tag="proj", bufs=2, name="pkv")
                    for kc in range(4):
                        nc.tensor.matmul(
                            pkv[:],
                            wb_sb[:, kc, mc * 128 : (mc + 1) * 128],
                            latent[:, kc, :],
                            start=(kc == 0),
                            stop=(kc == 3),
                        )
                    if mc % 2 == 0:
                        nc.scalar.copy(knope[h][:, tok], pkv[:])
                    else:
                        vT = work.tile([128, BLK], bf16, tag="vT", bufs=2)
                        nc.scalar.copy(vT[:], pkv[:])
                        pvt = ps.tile([128, BLK], bf16, tag="pvt", bufs=1, name="pvt")
                        for tt in range(4):
                            nc.tensor.transpose(
                                pvt[:, tt * 128 : (tt + 1) * 128],
                                vT[:, tt * 128 : (tt + 1) * 128],
                                ident[:],
                            )
                        nc.scalar.copy(vnat[h][:, qc * 4 : qc * 4 + 4, :], pvt[:])

            def stage_b(b, qc, qfT, knope, vnat, krope):
                """causal attention for one q-chunk, heads interleaved,
                one-kt software pipeline."""
                n_kt = 4 * (qc + 1)
                yaccs = [
                    ps.tile([VD, BLK], f32, tag=f"yacc{h}", bufs=1, name=f"yacc{h}")
                    for h in range(2)
                ]
                accs = [None, None]
                ests = {}
                for kt in range(n_kt + 1):
                    if kt < n_kt:
                        ks = slice(kt * 128, (kt + 1) * 128)
                        for h in range(2):
                            st = ps.tile(
                                [128, BLK], f32, tag=f"st{h}", bufs=1, name=f"st{h}"
                            )
                            nc.tensor.matmul(
                                st[:],
                                knope[h][:, ks],
                                qfT[:, h, :],
                                start=True,
                                stop=False,
                            )
                            nc.tensor.matmul(
                                st[:],
                                krope[:, ks],
                                qfT[0:64, 2 + h, :],
                                start=False,
                                stop=True,
                            )
                            est = work.tile([128, BLK], bf16, tag="est", bufs=4)
                            nc.scalar.activation(est[:], st[:], EXP)
                            if (kt + 1) * 128 > qc * BLK:
                                nc.gpsimd.affine_select(
                                    out=est[:],
                                    in_=est[:],
                                    compare_op=mybir.AluOpType.is_ge,
                                    fill=0.0,
                                    base=qc * BLK - kt * 128,
                                    pattern=[[1, BLK]],
                                    channel_multiplier=-1,
                                )
                            ests[(h, kt)] = est
                    if kt >= 1:
                        kp = kt - 1
                        for h in range(2):
                            e = ests.pop((h, kp))
                            nc.tensor.matmul(
                                yaccs[h][:],
                                vnat[h][:, kp, :],
                                e[:],
                                start=(kp == 0),
                                stop=(kp == n_kt - 1),
                            )
                            if kp == 0:
                                acc = work.tile(
                                    [128, BLK], f32, tag=f"acc{h}", bufs=2
                                )
                                accs[h] = acc
                                nc.vector.tensor_copy(acc[:], e[:])
                            else:
                                nc.vector.tensor_add(
                                    out=accs[h][:], in0=accs[h][:], in1=e[:]
                                )

                for h in range(2):
                    srow = stats[32 + 32 * h : 33 + 32 * h, :]
                    accb = work.tile([128, BLK], bf16, tag="accb", bufs=2)
                    nc.vector.tensor_copy(accb[:], accs[h][:])
                    nc.tensor.matmul(
                        srow,
                        ones[:],
                        accb[:],
                        start=True,
                        stop=True,
                    )
                    recro = work.tile([1, BLK], f32, tag="recro", bufs=2)
                    nc.vector.reciprocal(recro[:], srow)
                    recb16 = work.tile([1, BLK], bf16, tag="recb16", bufs=2)
                    nc.vector.tensor_copy(recb16[:], recro[:])
                    recb = work.tile([128, BLK], bf16, tag="recb", bufs=2)
                    nc.gpsimd.partition_broadcast(recb[:], recb16[:])
                    ysb = work.tile([VD, BLK], bf16, tag="ysb", bufs=2)
                    nc.vector.tensor_mul(out=ysb[:], in0=yaccs[h][:], in1=recb[:])
                    for jj in range(2):
                        nc.sync.dma_start(
                            y_in[b][qc * 2 + jj, h * VD : (h + 1) * VD, :],
                            ysb[:, jj * 256 : (jj + 1) * 256],
                        )

            def emit_a2a(b):
                nc.gpsimd.collective_compute(
                    "AllToAll",
                    mybir.AluOpType.bypass,
                    replica_groups=RG,
                    ins=[y_in[b].opt()],
                    outs=[y_out[b].opt()],
                )

            def emit_wo(b):
                """wo projection for this batch's gathered token slice.
                wt streams are issued before the a2a-gated gather so the
                weight transfers complete during the collective."""

                def load_wt(n):
                    pair = []
                    for half in range(2):
                        wt = wop.tile([128, 8, 512], bf16, tag="wt", bufs=4, name="wt")
                        nc.sync.dma_start(
                            wt[:],
                            wo_d[
                                half * 1024 : (half + 1) * 1024,
                                n * 512 : (n + 1) * 512,
                            ].rearrange("(kc p) c -> p kc c", p=128),
                        )
                        pair.append(wt)
                    return pair

                wts = {0: load_wt(0), 1: load_wt(1)}
                a2a = wop.tile([128, 16, 256], bf16, tag="a2a", bufs=1, name="a2a")
                nc.sync.dma_start(
                    a2a[:],
                    y_out[b][:].rearrange("c (two p) t -> p (c two) t", p=128),
                )
                for n in range(4):
                    if n + 2 < 4:
                        wts[n + 2] = load_wt(n + 2)
                    pouts = [
                        ps.tile([128, 512], f32, tag="proj", bufs=2, name="outp")
                        for _ in range(2)
                    ]
                    for kc in range(16):
                        for tt in range(2):
                            nc.tensor.matmul(
                                pouts[tt][:],
                                a2a[:, kc, tt * 128 : (tt + 1) * 128],
                                wts[n][kc // 8][:, kc % 8, :],
                                start=(kc == 0),
                                stop=(kc == 15),
                            )
                    for tt in range(2):
                        osb = wop.tile([128, 512], f32, tag="osb", bufs=2)
                        nc.scalar.copy(osb[:], pouts[tt][:])
                        nc.sync.dma_start(
                            out_d[
                                b, tt * 128 : (tt + 1) * 128, n * 512 : (n + 1) * 512
                            ],
                            osb[:],
                        )

            # ---- software-pipelined schedule ----
            pending_wo = None
            for b in range(B):
                knope, vnat, krope = alloc_kv()
                st_a = {}
                st_a[0] = stage_a1(b, 0)
                st_a[1] = stage_a1(b, 1)
                emit_ag(b, 0)
                st_a[2] = stage_a1(b, 2)
                st_a[3] = stage_a1(b, 3)
                emit_ag(b, 1)
                stage_a2(b, 0, knope, vnat, krope)
                stage_a2(b, 1, knope, vnat, krope)
                stage_b(b, 0, st_a[0], knope, vnat, krope)
                if pending_wo is not None:
                    emit_wo(pending_wo)
                    pending_wo = None
                stage_a2(b, 2, knope, vnat, krope)
                stage_b(b, 1, st_a[1], knope, vnat, krope)
                stage_a2(b, 3, knope, vnat, krope)
                stage_b(b, 2, st_a[2], knope, vnat, krope)
                stage_b(b, 3, st_a[3], knope, vnat, krope)
                emit_a2a(b)
                pending_wo = b
            emit_wo(pending_wo)

    nc.compile()
    return nc


def host_prep(x, wq, wkv_a, wkv_b, wo, kv_norm_w):
    scale = np.float32(QKD ** -0.5)
    inv = (1.0 / (10000.0 ** (np.arange(0, ROPE, 2, dtype=np.float32) / ROPE))).astype(
        np.float32
    )
    f = np.outer(np.arange(T, dtype=np.float32), inv)
    cos32 = np.cos(f).T.astype(np.float32)
    sin32 = np.sin(f).T.astype(np.float32)
    cos128 = np.ascontiguousarray(np.concatenate([cos32] * 4, 0)).astype(BF)
    sin128 = np.ascontiguousarray(np.concatenate([sin32] * 4, 0)).astype(BF)
    wkv_bw = (wkv_b * kv_norm_w[:, None]).astype(np.float32)
    xt = np.ascontiguousarray(x.reshape(B * T, D).T).astype(BF)
    wo_c = np.ascontiguousarray(wo).astype(BF)
    wq_r = wq.reshape(D, H, QKD)

    in_maps = []
    for c in range(NCORES):
        h0 = HPC * c
        w1 = np.concatenate(
            [
                wq_r[:, h0, :NOPE] * scale,
                wq_r[:, h0 + 1, :NOPE] * scale,
                wq_r[:, h0, NOPE:] * scale,
                wq_r[:, h0 + 1, NOPE:] * scale,
                wkv_a[:, c * KVS : (c + 1) * KVS],
            ],
            axis=1,
        ).astype(BF)
        wb = np.ascontiguousarray(
            wkv_bw[:, h0 * (NOPE + VD) : (h0 + 2) * (NOPE + VD)]
        ).astype(BF)
        in_maps.append(
            {
                "xt": xt,
                "w1": np.ascontiguousarray(w1),
                "wb": wb,
                "wo": wo_c,
                "cos": cos128,
                "sin": sin128,
            }
        )
    return in_maps


_NC = None


def kernel(x, wq, wkv_a, wkv_b, wo, kv_norm_w, _trace=False):
    global _NC
    if _NC is None:
        _NC = build_program()
    in_maps = host_prep(
        np.asarray(x, np.float32),
        np.asarray(wq, np.float32),
        np.asarray(wkv_a, np.float32),
        np.asarray(wkv_b, np.float32),
        np.asarray(wo, np.float32),
        np.asarray(kv_norm_w, np.float32),
    )
    res = run_bass_kernel_spmd(_NC, in_maps, list(range(NCORES)), trace=_trace)
    out = np.empty((B, T, D), np.float32)
    cw = T // NCORES
    for c in range(NCORES):
        oc = res.results[c]["out"]  # (B, 256, D)
        for b in range(B):
            out[b, c * cw : (c + 1) * cw, :] = oc[b]
    kernel.last_results = res
    return out
